# revision 10
# baseline (speedup 1.0000x reference)
"""GCN 2-layer (GCNConv 128->32 relu, GCNConv 32->7, log_softmax) on 8
trn2 NeuronCores, full inputs in / full output out.

Distribution: nodes sharded 8 ways by destination (12544/core after padding
100000 -> 100352). Per layer, each core aggregates messages for its own
dsts; the (projected, src-normalized) node table is replicated to every
core's HBM via AllGather between layers.

Per-core layer pipeline (all on device):
  - table view [25088, 128] bf16 = 4 nodes per 256-byte row so int16 gather
    indices cover all 100352 nodes without banking.
  - edges (+ explicit self-loops) dst-sorted into (dst-block x src%4 parity)
    segments, padded to a uniform tile count so one SPMD program fits every
    core; gpsimd dma_gather pulls 8192 rows/call into SBUF.
  - segment-sum via one-hot matmul: M[e,d] = (code[e] == d) built by a DVE
    is_equal over a whole chunk, then per 128-edge tile
    psum[dst,32] += M.T @ gathered[:, parity*32:+32] on the tensor engine.
  - layer ends: h1 = relu(dinv*acc + b1), g2 = h1*dinv (layer 1);
    y = log_softmax(dinv*acc @ W2 + b2) (layer 2), DVE/ACT + PE transpose.
"""
import sys

sys.path.insert(0, "/opt/trn_rl_repo")
import time

import numpy as np
import ml_dtypes

from concourse import bacc, bass, tile, bass_utils, mybir

BF16 = ml_dtypes.bfloat16
F32 = np.float32

NC = 8
N = 100000
NPAD = 100352
PER = NPAD // NC          # 12544
NBLK = PER // 128         # 98
NSEG = NBLK * 4           # segments per core (block x parity)
CH_T = 64                 # tiles per gather chunk
NI = CH_T * 128           # idxs per gather call (8192)

DEVICE_NS = [0]
_CACHE = {}


def _host_prep(x, edge_index, W1, b1, W2, b2):
    src = np.ascontiguousarray(edge_index[0]).astype(np.int64)
    dst = np.ascontiguousarray(edge_index[1]).astype(np.int64)
    deg = (np.bincount(dst, minlength=N) + 1.0).astype(F32)
    dinv = (1.0 / np.sqrt(deg)).astype(F32)
    dinv_pad = np.zeros(NPAD, F32)
    dinv_pad[:N] = dinv

    loop = np.arange(N, dtype=np.int64)
    src_all = np.concatenate([src, loop])
    dst_all = np.concatenate([dst, loop])

    core = dst_all // PER
    blk = (dst_all % PER) // 128
    par = src_all & 3
    code = dst_all % 128
    seg = ((core * NBLK + blk) * 4 + par).astype(np.int64)

    order = np.argsort(seg, kind="stable")
    seg_s = seg[order]
    src_s = src_all[order]
    code_s = code[order]

    cnt = np.bincount(seg_s, minlength=NC * NSEG).reshape(NC, NSEG)
    # per-segment tile count shared by all cores = max over the 8 cores
    seg_tiles = np.maximum(1, np.ceil(cnt.max(axis=0) / 128)).astype(np.int64)
    NT_real = int(seg_tiles.sum())             # real tiles per core
    NCH = int(np.ceil(NT_real / CH_T))
    NT = NCH * CH_T                            # padded tile count
    NSLOT = NT * 128
    seg_base = np.zeros(NSEG + 1, np.int64)
    np.cumsum(seg_tiles * 128, out=seg_base[1:])

    starts = np.zeros(NC * NSEG + 1, np.int64)
    np.cumsum(cnt.reshape(-1), out=starts[1:])
    local = np.arange(len(seg_s)) - starts[seg_s]
    slot = seg_base[seg_s % NSEG] + local      # slot within core
    core_s = seg_s // NSEG

    idx16 = np.zeros((NC, NSLOT), np.int16)
    codes = np.full((NC, NSLOT), 200.0, BF16)
    idx16[core_s, slot] = (src_s >> 2).astype(np.int16)
    codes[core_s, slot] = code_s.astype(BF16)

    # idx layout per chunk: idx j of chunk c -> [j%16, c*512 + j//16], x8 groups
    idx_strm = np.zeros((NC, 128, NSLOT // 16), np.int16)
    for c in range(NC):
        a = idx16[c].reshape(NCH, CH_T * 8, 16)
        a = a.transpose(0, 2, 1).reshape(NCH, 16, CH_T * 8)
        s16 = np.concatenate([a[i] for i in range(NCH)], axis=1)
        idx_strm[c] = np.tile(s16, (8, 1))
    code_strm = np.ascontiguousarray(
        codes.reshape(NC, NT, 128).transpose(0, 2, 1))

    xd = (x * dinv[:, None]).astype(BF16)
    xdT = np.zeros((NC, 128, PER), BF16)
    for c in range(NC):
        lo, hi = c * PER, min((c + 1) * PER, N)
        xdT[c, :, : hi - lo] = xd[lo:hi].T

    dinv_d = np.zeros((NC, 128, NBLK), F32)
    for c in range(NC):
        dinv_d[c] = dinv_pad[c * PER:(c + 1) * PER].reshape(NBLK, 128).T

    consts = dict(
        W1b=np.ascontiguousarray(W1.astype(BF16)),
        W2b=np.ascontiguousarray(W2.astype(BF16)),
        b1r=np.ascontiguousarray(np.tile(b1.astype(F32), (128, 1))),
        b2r=np.ascontiguousarray(np.tile(b2.astype(F32), (128, 1))),
        iota=np.ascontiguousarray(
            np.tile(np.arange(128, dtype=F32).astype(BF16), (128, 1))),
        ident=np.eye(128, dtype=F32).astype(BF16),
    )
    per_core = [
        dict(xdT=xdT[c], idxs=idx_strm[c], codes=code_strm[c],
             dinvd=dinv_d[c], **consts)
        for c in range(NC)
    ]
    return per_core, tuple(seg_tiles.tolist()), NCH


def _declare_io(nc, NT, NSLOT):
    dt = mybir.dt
    io = {}
    io["xdT"] = nc.dram_tensor("xdT", [128, PER], dt.bfloat16, kind="ExternalInput").ap()
    io["idxs"] = nc.dram_tensor("idxs", [128, NSLOT // 16], dt.int16, kind="ExternalInput").ap()
    io["codes"] = nc.dram_tensor("codes", [128, NT], dt.bfloat16, kind="ExternalInput").ap()
    io["dinvd"] = nc.dram_tensor("dinvd", [128, NBLK], dt.float32, kind="ExternalInput").ap()
    io["W1b"] = nc.dram_tensor("W1b", [128, 32], dt.bfloat16, kind="ExternalInput").ap()
    io["W2b"] = nc.dram_tensor("W2b", [32, 7], dt.bfloat16, kind="ExternalInput").ap()
    io["b1r"] = nc.dram_tensor("b1r", [128, 32], dt.float32, kind="ExternalInput").ap()
    io["b2r"] = nc.dram_tensor("b2r", [128, 7], dt.float32, kind="ExternalInput").ap()
    io["iota"] = nc.dram_tensor("iota", [128, 128], dt.bfloat16, kind="ExternalInput").ap()
    io["ident"] = nc.dram_tensor("ident", [128, 128], dt.bfloat16, kind="ExternalInput").ap()
    io["y"] = nc.dram_tensor("y", [PER, 7], dt.float32, kind="ExternalOutput").ap()
    return io


def _build(seg_tiles, NCH, reps=1):
    NT = NCH * CH_T
    NT_real = int(sum(seg_tiles))
    NSLOT = NT * 128
    # tile t -> (block, parity, first-of-block, last-of-block)
    tmeta = []
    for sid, ntile in enumerate(seg_tiles):
        blk, par = sid // 4, sid % 4
        for k in range(ntile):
            tmeta.append((blk, par,
                          par == 0 and k == 0,
                          par == 3 and k == ntile - 1))
    nc = bacc.Bacc("TRN2", target_bir_lowering=False, debug=False,
                   num_devices=NC)
    dt = mybir.dt
    io = _declare_io(nc, NT, NSLOT)
    xdT, idxs, codes, dinvd = io["xdT"], io["idxs"], io["codes"], io["dinvd"]
    W1b, W2b, b1r, b2r = io["W1b"], io["W2b"], io["b1r"], io["b2r"]
    iota, ident, y = io["iota"], io["ident"], io["y"]

    g1st = nc.dram_tensor("g1st", [PER, 32], dt.bfloat16, kind="Internal").ap()
    g1tab = nc.dram_tensor("g1tab", [NPAD // 4, 128], dt.bfloat16,
                           kind="Internal", addr_space="Shared").ap()
    g2st = nc.dram_tensor("g2st", [PER, 32], dt.bfloat16, kind="Internal").ap()
    g2tab = nc.dram_tensor("g2tab", [NPAD // 4, 128], dt.bfloat16,
                           kind="Internal", addr_space="Shared").ap()

    with tile.TileContext(nc) as tc:
        with tc.tile_pool(name="consts", bufs=1) as pc:
            w1_sb = pc.tile([128, 32], dt.bfloat16)
            w2_sb = pc.tile([32, 7], dt.bfloat16)
            b1_sb = pc.tile([128, 32], dt.float32)
            b2_sb = pc.tile([128, 7], dt.float32)
            iota_sb = pc.tile([128, 128], dt.bfloat16)
            id_sb = pc.tile([128, 128], dt.bfloat16)
            codes_sb = pc.tile([128, NT], dt.bfloat16)
            dinv_sb = pc.tile([128, NBLK], dt.float32)
            nc.sync.dma_start(w1_sb[:], W1b[:, :])
            nc.sync.dma_start(w2_sb[:], W2b[:, :])
            nc.sync.dma_start(b1_sb[:], b1r[:, :])
            nc.sync.dma_start(b2_sb[:], b2r[:, :])
            nc.sync.dma_start(iota_sb[:], iota[:, :])
            nc.sync.dma_start(id_sb[:], ident[:, :])
            nc.sync.dma_start(codes_sb[:], codes[:, :])
            nc.sync.dma_start(dinv_sb[:], dinvd[:, :])

            # ---- Phase 1: g1 shard = (xd @ W1) per own node, bf16 ----
            def phase1():
              with tc.tile_pool(name="p1", bufs=2) as p1, \
                   tc.tile_pool(name="p1ps", bufs=2, space="PSUM") as p1ps:
                  xdT_sb = p1.tile([128, PER], dt.bfloat16)
                  nc.sync.dma_start(xdT_sb[:], xdT[:, :])
                  for j in range(NBLK):
                      ps = p1ps.tile([128, 32], dt.float32, space="PSUM")
                      nc.tensor.matmul(
                          out=ps[:], lhsT=xdT_sb[:, j * 128:(j + 1) * 128],
                          rhs=w1_sb[:], start=True, stop=True)
                      gsb = p1.tile([128, 32], dt.bfloat16)
                      nc.vector.tensor_copy(gsb[:], ps[:])
                      nc.sync.dma_start(g1st[j * 128:(j + 1) * 128, :], gsb[:])
              nc.gpsimd.collective_compute(
                  "AllGather", mybir.AluOpType.bypass,
                  replica_groups=[list(range(NC))],
                  ins=[g1st[:, :]], outs=[g1tab[:, :]])

            def agg_layer(tab, finalize, out_dram, out_w, out_dt):
                with tc.tile_pool(name="pi", bufs=4) as pi, \
                     tc.tile_pool(name="pg", bufs=4) as pg, \
                     tc.tile_pool(name="pm", bufs=3) as pm, \
                     tc.tile_pool(name="pf", bufs=3) as pf, \
                     tc.tile_pool(name="po", bufs=1) as po, \
                     tc.tile_pool(name="pacc", bufs=2, space="PSUM") as pacc, \
                     tc.tile_pool(name="pfin", bufs=2, space="PSUM") as pfin:
                    obuf = po.tile([128, NBLK * out_w], out_dt, name="obuf")
                    ps_cur = [None]
                    for c in range(NCH):
                        i_sb = pi.tile([128, CH_T * 8], dt.int16)
                        nc.sync.dma_start(
                            i_sb[:], idxs[:, c * CH_T * 8:(c + 1) * CH_T * 8])
                        g_sb = pg.tile([128, CH_T, 128], dt.bfloat16)
                        nc.gpsimd.dma_gather(
                            g_sb[:], tab[:, :], i_sb[:],
                            num_idxs=NI, num_idxs_reg=NI, elem_size=128,
                            single_packet=False)
                        m_sb = pm.tile([128, CH_T, 128], dt.bfloat16)
                        nc.vector.tensor_tensor(
                            out=m_sb[:],
                            in0=codes_sb[:, c * CH_T:(c + 1) * CH_T]
                                .rearrange("p (t o) -> p t o", o=1)
                                .to_broadcast([128, CH_T, 128]),
                            in1=iota_sb[:]
                                .rearrange("p (o d) -> p o d", o=1)
                                .to_broadcast([128, CH_T, 128]),
                            op=mybir.AluOpType.is_equal)
                        for t64 in range(CH_T):
                            t = c * CH_T + t64
                            if t >= NT_real:
                                break
                            b, q, first, last = tmeta[t]
                            if first:
                                ps_cur[0] = pacc.tile(
                                    [128, 32], dt.float32, space="PSUM",
                                    name="acc")
                            nc.tensor.matmul(
                                out=ps_cur[0][:],
                                lhsT=m_sb[:, t64, :],
                                rhs=g_sb[:, t64, q * 32:(q + 1) * 32],
                                start=first, stop=last)
                            if last:
                                finalize(b, ps_cur[0], pf, pfin, obuf)
                    nc.sync.dma_start(
                        out_dram[:, :].rearrange("(b a) w -> a b w", a=128),
                        obuf[:].rearrange("p (b w) -> p b w", w=out_w))

            def fin1(b, ps, pf, pfin, obuf):
                t0 = pf.tile([128, 32], dt.float32)
                nc.vector.tensor_tensor(
                    out=t0[:], in0=ps[:],
                    in1=dinv_sb[:, b:b + 1].to_broadcast([128, 32]),
                    op=mybir.AluOpType.mult)
                nc.vector.tensor_tensor(out=t0[:], in0=t0[:], in1=b1_sb[:],
                                        op=mybir.AluOpType.add)
                nc.vector.tensor_scalar(
                    out=t0[:], in0=t0[:], scalar1=0.0, scalar2=None,
                    op0=mybir.AluOpType.max)
                nc.vector.tensor_tensor(
                    out=obuf[:, b * 32:(b + 1) * 32], in0=t0[:],
                    in1=dinv_sb[:, b:b + 1].to_broadcast([128, 32]),
                    op=mybir.AluOpType.mult)


            def fin2(b, ps, pf, pfin, obuf):
                s_sb = pf.tile([128, 32], dt.bfloat16)
                nc.vector.tensor_tensor(
                    out=s_sb[:], in0=ps[:],
                    in1=dinv_sb[:, b:b + 1].to_broadcast([128, 32]),
                    op=mybir.AluOpType.mult)
                tps = pfin.tile([32, 128], dt.bfloat16, space="PSUM")
                nc.tensor.transpose(out=tps[:], in_=s_sb[:], identity=id_sb[:])
                st_sb = pf.tile([32, 128], dt.bfloat16)
                nc.vector.tensor_copy(st_sb[:], tps[:])
                yps = pfin.tile([128, 7], dt.float32, space="PSUM")
                nc.tensor.matmul(out=yps[:], lhsT=st_sb[:], rhs=w2_sb[:],
                                 start=True, stop=True)
                y0 = pf.tile([128, 7], dt.float32)
                nc.vector.tensor_tensor(out=y0[:], in0=yps[:], in1=b2_sb[:],
                                        op=mybir.AluOpType.add)
                mx = pf.tile([128, 1], dt.float32)
                nc.vector.tensor_reduce(out=mx[:], in_=y0[:],
                                        axis=mybir.AxisListType.X,
                                        op=mybir.AluOpType.max)
                ysh = pf.tile([128, 7], dt.float32)
                nc.vector.tensor_tensor(
                    out=ysh[:], in0=y0[:],
                    in1=mx[:].to_broadcast([128, 7]),
                    op=mybir.AluOpType.subtract)
                ex = pf.tile([128, 7], dt.float32)
                nc.scalar.activation(ex[:], ysh[:],
                                     func=mybir.ActivationFunctionType.Exp)
                sm = pf.tile([128, 1], dt.float32)
                nc.vector.tensor_reduce(out=sm[:], in_=ex[:],
                                        axis=mybir.AxisListType.X,
                                        op=mybir.AluOpType.add)
                ls = pf.tile([128, 1], dt.float32)
                nc.scalar.activation(ls[:], sm[:],
                                     func=mybir.ActivationFunctionType.Ln)
                nc.vector.tensor_tensor(
                    out=obuf[:, b * 7:(b + 1) * 7], in0=ysh[:],
                    in1=ls[:].to_broadcast([128, 7]),
                    op=mybir.AluOpType.subtract)

            for _rep in range(reps):
                phase1()
                agg_layer(g1tab, fin1, g2st, 32, dt.bfloat16)
                nc.gpsimd.collective_compute(
                    "AllGather", mybir.AluOpType.bypass,
                    replica_groups=[list(range(NC))],
                    ins=[g2st[:, :]], outs=[g2tab[:, :]])
                agg_layer(g2tab, fin2, y, 7, dt.float32)
    nc.compile()
    return nc


def _build_null(T_bp, NCH):
    """Same I/O signature, trivial body — measures launch overhead."""
    NT = NCH * CH_T
    NSLOT = NT * 128
    nc = bacc.Bacc("TRN2", target_bir_lowering=False, debug=False,
                   num_devices=NC)
    dt = mybir.dt
    io = _declare_io(nc, NT, NSLOT)
    with tile.TileContext(nc) as tc:
        with tc.tile_pool(name="p", bufs=1) as p:
            t = p.tile([128, 7], dt.float32)
            nc.sync.dma_start(t[:], io["b2r"][:, :])
            nc.sync.dma_start(io["y"][0:128, :], t[:])
    nc.compile()
    return nc


def _make_runner(nc, n_cores=NC):
    """jit once; device-resident inputs; repeatable execution."""
    import jax
    from jax.sharding import Mesh, PartitionSpec
    from jax.experimental.shard_map import shard_map
    from concourse import bass2jax

    bass2jax.install_neuronx_cc_hook()
    partition_name = nc.partition_id_tensor.name if nc.partition_id_tensor else None
    in_names, out_names, out_avals = [], [], []
    for alloc in nc.m.functions[0].allocations:
        if not isinstance(alloc, mybir.MemoryLocationSet):
            continue
        name = alloc.memorylocations[0].name
        if alloc.kind == "ExternalInput":
            if name != partition_name:
                in_names.append(name)
        elif alloc.kind == "ExternalOutput":
            out_names.append(name)
            out_avals.append(jax.core.ShapedArray(
                tuple(alloc.tensor_shape), mybir.dt.np(alloc.dtype)))
    n_params = len(in_names)
    all_names = in_names + out_names
    if partition_name is not None:
        all_names = all_names + [partition_name]

    def _body(*args):
        operands = list(args)
        if partition_name is not None:
            operands.append(bass2jax.partition_id_tensor())
        outs = bass2jax._bass_exec_p.bind(
            *operands,
            out_avals=tuple(out_avals),
            in_names=tuple(all_names),
            out_names=tuple(out_names),
            lowering_input_output_aliases=(),
            sim_require_finite=True,
            sim_require_nnan=True,
            nc=nc)
        return tuple(outs)

    devices = jax.devices()[:n_cores]
    mesh = Mesh(np.asarray(devices), ("core",))
    in_specs = (PartitionSpec("core"),) * (n_params + len(out_names))
    out_specs = (PartitionSpec("core"),) * len(out_names)
    fn = jax.jit(shard_map(_body, mesh=mesh, in_specs=in_specs,
                           out_specs=out_specs, check_rep=False),
                 keep_unused=True)

    def prep(in_maps):
        arrs = []
        for name in in_names:
            a = np.concatenate([np.asarray(m[name]) for m in in_maps], axis=0)
            arrs.append(jax.device_put(a))
        for av in out_avals:
            z = np.zeros((n_cores * av.shape[0], *av.shape[1:]), av.dtype)
            arrs.append(jax.device_put(z))
        jax.block_until_ready(arrs)
        return arrs

    def run(arrs):
        import jax
        outs = fn(*arrs)
        jax.block_until_ready(outs)
        return outs

    return prep, run


def kernel(x, edge_index, W1, b1, W2, b2):
    x = np.asarray(x, F32)
    W1 = np.asarray(W1, F32)
    b1 = np.asarray(b1, F32)
    W2 = np.asarray(W2, F32)
    b2 = np.asarray(b2, F32)
    per_core, seg_tiles, NCH = _host_prep(x, edge_index, W1, b1, W2, b2)
    key = ("main", seg_tiles, NCH)
    if key not in _CACHE:
        _CACHE[key] = _build(seg_tiles, NCH)
    nc = _CACHE[key]
    res = bass_utils.run_bass_kernel_spmd(nc, per_core, list(range(NC)))
    _CACHE["last"] = (seg_tiles, NCH, per_core)
    out = np.concatenate(
        [np.asarray(res.results[c]["y"]) for c in range(NC)], axis=0)
    return np.ascontiguousarray(out[:N]).astype(F32)


def measure_hw_ns(iters=8, reps=5, reps_lo=2):
    """On-device time of one inference, from the slope between programs
    that run the pipeline `reps` vs `reps_lo` times (identical I/O and
    launch overhead; slope removes it)."""
    assert "last" in _CACHE, "call kernel() first"
    seg_tiles, NCH, per_core = _CACHE["last"]
    key_l = ("reps", seg_tiles, NCH, reps_lo)
    if key_l not in _CACHE:
        _CACHE[key_l] = _build(seg_tiles, NCH, reps=reps_lo)
    nc_1 = _CACHE[key_l]
    key_r = ("reps", seg_tiles, NCH, reps)
    if key_r not in _CACHE:
        _CACHE[key_r] = _build(seg_tiles, NCH, reps=reps)
    nc_r = _CACHE[key_r]

    def bench(nc):
        prep, run = _make_runner(nc)
        arrs = prep(per_core)
        run(arrs)  # warmup (includes NEFF compile+load)
        ts = []
        for _ in range(iters):
            t0 = time.perf_counter()
            run(arrs)
            ts.append(time.perf_counter() - t0)
        ts.sort()
        return ts

    ts_r = bench(nc_r)
    ts_1 = bench(nc_1)
    med_r = ts_r[len(ts_r) // 2]
    med_1 = ts_1[len(ts_1) // 2]
    hw = max(0.0, (med_r - med_1) / (reps - reps_lo))
    DEVICE_NS[0] = int(hw * 1e9)
    return dict(hw_ns=DEVICE_NS[0], full_s=ts_r, null_s=ts_1)
